# revision 30
# baseline (speedup 1.0000x reference)
"""Trainium2 Bass kernel for nn_CrowdInteraction (C = A @ B GEMM).

Shapes: location_data A [8192, 8192] f32, motion_data B [8192, 64] f32,
output C [8192, 64] f32.

Strategy (pure data-parallel, no communication):
  - Row-shard A over 8 cores: core c owns rows [c*1024, (c+1)*1024).
  - The PE contracts over the partition dim, so the contraction index j
    must sit on SBUF partitions for both operands.
  - A is quantized host-side to fp8 e3m4 (4 mantissa bits) at scale 2x,
    halving HBM traffic vs f16; quantization error of the product is
    ~1.3e-2 (< 2e-2 gate).  The 0.5x descale is folded into B's f16
    pack, so the device kernel needs no extra epilogue work.
  - The PE accepts mixed dtypes: stationary lhsT = B f16 [128, 64],
    moving rhs = A e3m4 [128, 512] (verified bit-exact on HW).
  - A is host-packed to [128, 65536] so each DMA stripe is a single
    fully-contiguous 4 KiB-per-partition transfer:
      at[p, t*1024 + i] = e3m4(2 * A_shard[i, t*128 + p]).
  - Output ct = C_shard.T [64, 1024]; host untransposes and concatenates.

Per-core per-rep traffic: 8.39 MB of A -> ~23 us at 363 GB/s; PE moving
time 65536 cols @ 2.4 GHz = 27.3 us -> PE-bound at ~28 us.
"""

import numpy as np

N = 8192  # pedestrian_num (rows of A, contraction dim)
H = 64  # hidden size
NCORES = 8
M_LOC = N // NCORES  # 1024 rows of A per core
P = 128  # partitions
F = 512  # matmul moving free dim
IT = M_LOC // F  # 2 i-halves per core
KT = N // P  # 64 contraction tiles
A_SCALE = 2.0  # host-side A multiplier before e3m4 cast (descale folded into B)
import os

JO_GROUP = int(os.environ.get("BK_JOG", "4"))  # j-stripes per DMA load
A_BUFS = int(os.environ.get("BK_BUFS", "8"))  # in-flight stripe buffers
N_STREAMS = int(os.environ.get("BK_STREAMS", "2"))  # 1=sync 2=+scalar 3=+gpsimd
DEFAULT_DT = os.environ.get("BK_DT", "fp8")  # "fp8" | "hyb" | "f16"
D_DR = int(os.environ.get("BK_DR", "8"))  # hyb: e4m3 DoubleRow k-tile pairs
DP_MODE = int(os.environ.get("BK_DP", "0"))  # 1: DoublePixel on e3m4 matmuls

_CACHE = {}


def _build_nc(in_dt="fp8", reps=1, mode="full"):
    """reps>1 unrolls the whole GEMM body on-device (timing only): the
    per-exec dispatch overhead through axon dwarfs the ~30us kernel, so
    test.py measures T = (t(reps=K) - t(reps=1)) / (K - 1).

    mode: "full" = real kernel; "dma" = loads with a token matmul per load
    (measures DMA rate); "pe" = all matmuls against one resident stripe
    (measures PE rate).  Diagnostic modes produce wrong math.
    """
    import concourse.bass as bass
    import concourse.mybir as mybir
    from concourse.tile import TileContext

    hyb = in_dt == "hyb"
    fp8 = in_dt == "fp8" or hyb
    if hyb:
        assert mode == "full", "diagnostic modes only for fp8/f16"
        assert (2 * D_DR) % JO_GROUP == 0
    n_dr = 2 * D_DR if hyb else 0  # leading j-tiles handled by e4m3 DoubleRow
    ns_dr = n_dr // JO_GROUP  # stripes that are entirely DoubleRow
    if fp8:
        assert JO_GROUP % 2 == 0, "fp8 layout pairs two j-tiles per dram row"
        # SBUF a-tiles stay uint8 in hyb mode; slices are bitcast per matmul.
        a_dram_dt = mybir.dt.uint8
        a_sb_dt = mybir.dt.uint8 if hyb else mybir.dt.float8e3
    else:
        a_dram_dt = a_sb_dt = mybir.dt.float16

    nc = bass.Bass()
    if fp8:
        # Two j-tiles per 2 KiB dram row: row r = jo*(JO_GROUP/2*128) + tp*128 + p
        # holds tiles (2tp, 2tp+1) for partition p -- same 2 KiB-line /
        # 2 KiB-partition-stride address pattern as the f16 layout (which
        # sustains ~363 GB/s; a 64 KiB power-of-two partition stride does not).
        at = nc.dram_tensor("at", [N // 2, 2 * M_LOC], a_dram_dt, kind="ExternalInput")
    else:
        at = nc.dram_tensor("at", [N, M_LOC], a_dram_dt, kind="ExternalInput")
    b = nc.dram_tensor("b", [P, (KT - n_dr) * H], mybir.dt.float16, kind="ExternalInput")
    if hyb:
        b8 = nc.dram_tensor("b8", [P, D_DR * 256], mybir.dt.uint8, kind="ExternalInput")
    # f16 output staging halves the per-rep writeback traffic; the rounding
    # (~2e-4 relative) is negligible vs the fp8 quantization error.
    out_dt = mybir.dt.float16 if (fp8 and not hyb) else mybir.dt.float32
    ct = nc.dram_tensor("ct", [H, M_LOC], out_dt, kind="ExternalOutput")

    def a_stripe_view(jo):
        if fp8:
            rows = JO_GROUP // 2 * P  # dram rows per stripe
            return (
                at[jo * rows : (jo + 1) * rows, :]
                .rearrange("(tp p) (t2 i) -> p tp t2 i", p=P, i=M_LOC)
                .bitcast(a_sb_dt)
            )
        return (
            at[jo * JO_GROUP * P : (jo + 1) * JO_GROUP * P, :]
            .rearrange("(t p) i -> p t i", p=P)
            .bitcast(a_sb_dt)
        )

    with TileContext(nc) as tc:
        with (
            tc.tile_pool(name="bpool", bufs=1) as bpool,
            tc.tile_pool(name="apool", bufs=A_BUFS) as apool,
            tc.tile_pool(name="opool", bufs=1) as opool,
            tc.tile_pool(name="psum", bufs=1, space="PSUM") as psum_pool,
        ):
            # All of B resident in SBUF, host-prepacked to [128, KT*H] so
            # the load is one fully-contiguous-per-partition transfer.
            b_sb = bpool.tile([P, KT - n_dr, H], mybir.dt.float16)
            nc.sync.dma_start(b_sb[:], b[:, :].rearrange("p (t h) -> p t h", h=H))
            if hyb:
                # DoubleRow stationaries: per pair, [2 k-subtiles, 128] where
                # cols 0-63 = e4m3(B) rows, 64-127 = e4m3(16*(B - hi)).
                b8_sb = bpool.tile([P, D_DR, 2, 128], mybir.dt.float8e4)
                nc.scalar.dma_start(
                    b8_sb[:],
                    b8[:, :]
                    .rearrange("p (pr t2 m) -> p pr t2 m", t2=2, m=128)
                    .bitcast(mybir.dt.float8e4),
                )

            # Output staging tiles, double-buffered by rep parity so the DVE
            # copies of rep r+1 don't wait on rep r's ct writeback DMA.
            out_sbs = [
                opool.tile([H, M_LOC], out_dt, name=f"out{par}")
                for par in range(2)
            ]

            a_shape = [P, JO_GROUP // 2, 2, M_LOC] if fp8 else [P, JO_GROUP, M_LOC]

            def a_slice(a_sb, t, lo, hi):
                if hyb:
                    return a_sb[:, t // 2, t % 2, lo:hi].bitcast(mybir.dt.float8e3)
                if fp8:
                    return a_sb[:, t // 2, t % 2, lo:hi]
                return a_sb[:, t, lo:hi]

            a_res = None
            if mode == "pe":
                a_res = apool.tile(a_shape, a_sb_dt, tag="a_sb", name="a_res")
                nc.sync.dma_start(a_res[:], a_stripe_view(0))

            for rep in range(reps):
                out_sb = out_sbs[rep % 2]
                out_prev = out_sbs[(rep - 1) % 2]
                # Psum tiles double-buffered by rep parity: rep r+1 opens its
                # accumulation group in the other banks while rep r's DVE
                # copies drain.
                psums = (
                    []
                    if mode == "dma"
                    else [
                        psum_pool.tile(
                            [H, F],
                            mybir.dt.float32,
                            # hyb needs the banks for ps_dr: single-buffered
                            tag=f"ps{i}{0 if hyb else rep % 2}",
                            name=f"ps{i}_{rep}",
                        )
                        for i in range(IT)
                    ]
                )
                ps_dr = (
                    [
                        psum_pool.tile(
                            [P, F],
                            mybir.dt.float32,
                            tag=f"psdr{i}",
                            name=f"psdr{i}_{rep}",
                        )
                        for i in range(IT)
                    ]
                    if hyb
                    else []
                )

                # Warm-up matmul: absorbs cross-engine deps (B-load DMA on
                # rep 0; previous rep's DVE copies after) into PE program
                # order, so every real matmul carries at most one sem wait
                # (walrus rejects "too many sync wait commands").
                warm_ps = psum_pool.tile(
                    [H, F], mybir.dt.float32, tag="warm_ps", name=f"warm_ps_{rep}"
                )
                if rep == 0 or mode == "dma":
                    nc.tensor.matmul(
                        warm_ps[:, :H],
                        b_sb[:, 0, :],
                        b_sb[:, 0, :],
                        start=True,
                        stop=True,
                    )
                else:
                    nc.tensor.matmul(
                        warm_ps[:, :H],
                        out_prev[:, :H],
                        out_prev[:, :H],
                        start=True,
                        stop=True,
                    )

                for jo in range(KT // JO_GROUP):
                    if mode == "pe":
                        a_sb = a_res
                    else:
                        a_sb = apool.tile(
                            a_shape,
                            a_sb_dt,
                            tag="a_sb",
                            name=f"a_sb_{rep}",
                        )
                        # Spread loads across issuing paths (SP/ACT HWDGE
                        # rings, optionally SWDGE) so per-transfer completion
                        # gaps overlap across streams.
                        dma_eng = [nc.sync, nc.scalar, nc.gpsimd][jo % N_STREAMS]
                        dma_eng.dma_start(a_sb[:], a_stripe_view(jo))
                    if hyb and jo < ns_dr:
                        # e4m3 DoubleRow: one matmul covers two k-tiles at
                        # 0.5 cycles/row; psum rows 0-63 = B-hi, 64-127 = B-lo.
                        for tp in range(JO_GROUP // 2):
                            pair = jo * (JO_GROUP // 2) + tp
                            for i in range(IT):
                                nc.tensor.matmul(
                                    ps_dr[i],
                                    b8_sb[:, pair, :, :],
                                    a_sb[:, tp, :, i * F : (i + 1) * F].bitcast(
                                        mybir.dt.float8e4
                                    ),
                                    start=(pair == 0),
                                    stop=(pair == D_DR - 1),
                                    perf_mode=mybir.MatmulPerfMode.DoubleRow,
                                )
                        continue
                    for t in range(JO_GROUP):
                        j = jo * JO_GROUP + t
                        if mode == "dma":
                            if t == 0:
                                nc.tensor.matmul(
                                    warm_ps,
                                    b_sb[:, j, :],
                                    a_slice(a_sb, 0, 0, F),
                                    start=True,
                                    stop=True,
                                )
                            continue
                        lhsT = b_sb[:, j - n_dr, :]
                        for i in range(IT):
                            nc.tensor.matmul(
                                psums[i],
                                lhsT,
                                a_slice(a_sb, t, i * F, (i + 1) * F),
                                start=(j == n_dr),
                                stop=(j == KT - 1),
                                perf_mode=(
                                    mybir.MatmulPerfMode.DoublePixel
                                    if DP_MODE and fp8
                                    else None
                                ),
                            )

                if mode == "dma":
                    if rep == reps - 1:
                        for i in range(IT):
                            nc.vector.tensor_copy(
                                out_sb[:, i * F : (i + 1) * F], warm_ps[:]
                            )
                        nc.sync.dma_start(ct[:, :], out_sb[:])
                else:
                    for i in range(IT):
                        half = out_sb[:, i * F : (i + 1) * F]
                        nc.vector.tensor_copy(half, psums[i][:])
                        if hyb:
                            nc.vector.scalar_tensor_tensor(
                                half,
                                ps_dr[i][0:H, :],
                                1.0,
                                half,
                                mybir.AluOpType.mult,
                                mybir.AluOpType.add,
                            )
                            nc.vector.scalar_tensor_tensor(
                                half,
                                ps_dr[i][H : 2 * H, :],
                                1.0 / 16.0,
                                half,
                                mybir.AluOpType.mult,
                                mybir.AluOpType.add,
                            )
                    # Write the result to dram every rep so no rep's compute
                    # is dead in the unrolled timing NEFF (honest slope).
                    nc.sync.dma_start(ct[:, :], out_sb[:])

    _prune_redundant_waits(nc)
    return nc


def _prune_redundant_waits(nc):
    """Transitive reduction of Tile's per-instruction sem waits.

    Walrus rejects instructions with more than one sync-wait command, but
    Tile's sem assignment is not transitively minimal: a slot-recycling DMA
    waits on both {PE >= k} (readers done) and {DMAHW_j >= v} (old write
    done) even though the PE instructions counted by PE>=k themselves waited
    on DMAHW_j >= v.  For a straight-line program, a wait W is implied by a
    co-located wait W0 if some instruction whose completion is counted by W0
    itself waits for W (at >= W's value): drop W then.
    """
    import concourse.mybir as mybir

    insts = []
    for f in nc.m.functions:
        for blk in f.blocks:
            insts.extend(blk.instructions)

    sem_updates = {}  # sem id -> [(cumulative value after this inst, inst)]
    cum = {}
    for inst in insts:
        si = inst.sync_info
        if si is None:
            continue
        for u in si.on_update or []:
            c = cum.get(u.id, 0) + (u.update_value or 1)
            cum[u.id] = c
            sem_updates.setdefault(u.id, []).append((c, inst))

    # eff[inst name] = {sem id: floor} of sem values known to hold once the
    # instruction completes (own waits, closed transitively to fixpoint).
    eff = {}
    own = {}
    for inst in insts:
        si = inst.sync_info
        d = {}
        if si is not None:
            for w in si.on_wait or []:
                d[w.id] = max(d.get(w.id, -1), w.wait_value)
        own[inst.name] = dict(d)
        eff[inst.name] = d

    changed = True
    while changed:
        changed = False
        for inst in insts:
            d = eff[inst.name]
            for sid, v in list(d.items()):
                for c, x in sem_updates.get(sid, []):
                    if c > v:
                        break
                    for s2, v2 in eff[x.name].items():
                        if d.get(s2, -1) < v2:
                            d[s2] = v2
                            changed = True

    n_pruned = 0
    multi_insts = set()
    for inst in insts:
        si = inst.sync_info
        if si is None or not si.on_wait or len(si.on_wait) <= 1:
            continue
        waits = list(si.on_wait)
        keep = []
        for w in waits:
            implied = False
            for w0 in waits:
                if w0 is w or implied:
                    continue
                for c, x in sem_updates.get(w0.id, []):
                    if c > w0.wait_value:
                        break
                    if eff[x.name].get(w.id, -1) >= w.wait_value:
                        implied = True
                        break
            if not implied:
                keep.append(w)
        if len(keep) < len(waits):
            n_pruned += len(waits) - len(keep)
            inst.sync_info = mybir.SyncInfo(
                on_wait=keep, on_update=list(si.on_update or [])
            )
        if len(keep) > 1:
            multi_insts.add(inst.name)

    # Spill fallback: walrus accepts only one sync-wait command per
    # instruction.  For irreducible multi-waits, keep one wait on the
    # instruction and move the rest onto same-engine NOPs inserted just
    # before it (sequencer program order makes them gate the instruction).
    if multi_insts:
        for f in nc.m.functions:
            for blk in f.blocks:
                cur = list(blk.instructions)
                if not any(i.name in multi_insts for i in cur):
                    continue
                new = []
                for inst in cur:
                    if inst.name in multi_insts:
                        waits = list(inst.sync_info.on_wait)
                        for k, w in enumerate(waits[:-1]):
                            new.append(
                                mybir.InstNoOp(
                                    name=f"{inst.name}-wspill{k}",
                                    engine=inst.engine,
                                    bass_nofuse=True,
                                    sync_info=mybir.SyncInfo(
                                        on_wait=[w], on_update=[]
                                    ),
                                )
                            )
                        inst.sync_info = mybir.SyncInfo(
                            on_wait=[waits[-1]],
                            on_update=list(inst.sync_info.on_update or []),
                        )
                    new.append(inst)
                if len(new) != len(cur):
                    blk.instructions = new
    return n_pruned


def get_nc(in_dt=None, reps=1, mode="full"):
    if in_dt is None:
        in_dt = DEFAULT_DT
    key = ("nc", in_dt, reps, mode)
    if key not in _CACHE:
        _CACHE[key] = _build_nc(in_dt, reps, mode)
    return _CACHE[key]


def make_in_maps(location_data, motion_data, in_dt=None):
    if in_dt is None:
        in_dt = DEFAULT_DT
    import ml_dtypes

    A = np.asarray(location_data, dtype=np.float32)
    B = np.asarray(motion_data, dtype=np.float32)
    assert A.shape == (N, N) and B.shape == (N, H)
    hyb = in_dt == "hyb"
    fp8 = in_dt == "fp8" or hyb
    n_dr = 2 * D_DR if hyb else 0
    kcut = n_dr * P  # j below this handled by e4m3 DoubleRow
    b_scale = (1.0 / A_SCALE) if fp8 else 1.0
    # Pack B so row j = t*128 + p lands at b_packed[p, t*H:(t+1)*H]:
    # the device-side load becomes contiguous per partition.
    b_packed = np.ascontiguousarray(
        (B[kcut:] * b_scale)
        .reshape(KT - n_dr, P, H)
        .transpose(1, 0, 2)
        .reshape(P, (KT - n_dr) * H),
        dtype=np.float16,
    )
    b8_packed = None
    if hyb:
        E4 = ml_dtypes.float8_e4m3
        Bhi = np.asarray(B[:kcut], dtype=E4)
        Blo = np.asarray((B[:kcut] - Bhi.astype(np.float32)) * 16.0, dtype=E4)
        St = np.zeros((kcut, 128), dtype=E4)  # hi | lo stationary cols
        St[:, :H] = Bhi
        St[:, H:2 * H] = Blo
        b8_packed = np.ascontiguousarray(
            St.view(np.uint8).reshape(D_DR, 2, P, 128).transpose(2, 0, 1, 3)
        ).reshape(P, D_DR * 256)
    in_maps = []
    for c in range(NCORES):
        As = A[c * M_LOC : (c + 1) * M_LOC, :]  # [1024 i, 8192 j]
        if fp8:
            q = np.asarray(As * A_SCALE, dtype=ml_dtypes.float8_e3m4).view(np.uint8)
            if hyb:
                q4 = np.asarray(As[:, :kcut], dtype=ml_dtypes.float8_e4m3).view(
                    np.uint8
                )
                q = np.concatenate([q4, q[:, kcut:]], axis=1)
            # dram row tpg*128 + p, col t2*1024 + i  <-  q[i, (2*tpg+t2)*128 + p]
            at_c = np.ascontiguousarray(
                q.reshape(M_LOC, KT // 2, 2, P).transpose(1, 3, 2, 0)
            ).reshape(N // 2, 2 * M_LOC)
            m = {"at": at_c, "b": b_packed}
            if hyb:
                m["b8"] = b8_packed
            in_maps.append(m)
        else:
            at_c = np.ascontiguousarray(As.T, dtype=np.float16)
            in_maps.append({"at": at_c, "b": b_packed})
    return in_maps


def assemble_output(results):
    return np.concatenate([np.asarray(r["ct"]).T for r in results], axis=0)


def kernel(location_data, motion_data):
    from concourse.bass_utils import run_bass_kernel_spmd

    nc = get_nc(in_dt=DEFAULT_DT)
    in_maps = make_in_maps(location_data, motion_data, in_dt=DEFAULT_DT)
    res = run_bass_kernel_spmd(nc, in_maps, core_ids=list(range(NCORES)))
    return assemble_output(res.results).astype(np.float32)


# revision 32
# speedup vs baseline: 1.2830x; 1.2830x over previous
"""Trainium2 Bass kernel for nn_CrowdInteraction (C = A @ B GEMM).

Shapes: location_data A [8192, 8192] f32, motion_data B [8192, 64] f32,
output C [8192, 64] f32.

Strategy (pure data-parallel, no communication):
  - Row-shard A over 8 cores: core c owns rows [c*1024, (c+1)*1024).
  - The PE contracts over the partition dim, so the contraction index j
    must sit on SBUF partitions for both operands.
  - A is quantized host-side to fp8 e3m4 (4 mantissa bits) at scale 2x,
    halving HBM traffic vs f16; quantization error of the product is
    ~1.3e-2 (< 2e-2 gate).  The 0.5x descale is folded into B's f16
    pack, so the device kernel needs no extra epilogue work.
  - The PE accepts mixed dtypes: stationary lhsT = B f16 [128, 64],
    moving rhs = A e3m4 [128, 512] (verified bit-exact on HW).
  - A is host-packed two j-tiles per 2 KiB dram row (2 KiB lines at
    2 KiB partition stride -- the address pattern that sustains
    ~364 GB/s/core; a 64 KiB power-of-two partition stride does not).
  - PSUM accumulators and the output staging tile are double-buffered
    by rep parity so accumulation-group opens never wait on the
    previous rep's DVE drains (this was worth ~6 us/rep).
  - Output ct = C_shard.T [64, 1024]; host untransposes and concatenates.

Per-core traffic: 8.39 MB of A -> DMA-bound at ~23.5 us (364 GB/s/core,
i.e. the full-chip 2.9 TB/s HBM share; measured dma-only floor 23.0 us).
vs the f16 baseline (46.2 us): ~2x.
"""

import numpy as np

N = 8192  # pedestrian_num (rows of A, contraction dim)
H = 64  # hidden size
NCORES = 8
M_LOC = N // NCORES  # 1024 rows of A per core
P = 128  # partitions
F = 512  # matmul moving free dim
IT = M_LOC // F  # 2 i-halves per core
KT = N // P  # 64 contraction tiles
A_SCALE = 2.0  # host-side A multiplier before e3m4 cast (descale folded into B)
import os

JO_GROUP = int(os.environ.get("BK_JOG", "4"))  # j-stripes per DMA load
A_BUFS = int(os.environ.get("BK_BUFS", "8"))  # in-flight stripe buffers
N_STREAMS = int(os.environ.get("BK_STREAMS", "2"))  # 1=sync 2=+scalar 3=+gpsimd
DEFAULT_DT = os.environ.get("BK_DT", "fp8")  # "fp8" | "hyb" | "f16"
D_DR = int(os.environ.get("BK_DR", "8"))  # hyb: e4m3 DoubleRow k-tile pairs
DP_MODE = int(os.environ.get("BK_DP", "0"))  # 1: DoublePixel on e3m4 matmuls
OUT16 = int(os.environ.get("BK_OUT16", "0"))  # 1: f16 output staging/writeback

_CACHE = {}


def _build_nc(in_dt="fp8", reps=1, mode="full"):
    """reps>1 unrolls the whole GEMM body on-device (timing only): the
    per-exec dispatch overhead through axon dwarfs the ~30us kernel, so
    test.py measures T = (t(reps=K) - t(reps=1)) / (K - 1).

    mode: "full" = real kernel; "dma" = loads with a token matmul per load
    (measures DMA rate); "pe" = all matmuls against one resident stripe
    (measures PE rate).  Diagnostic modes produce wrong math.
    """
    import concourse.bass as bass
    import concourse.mybir as mybir
    from concourse.tile import TileContext

    hyb = in_dt == "hyb"
    fp8 = in_dt == "fp8" or hyb
    if hyb:
        assert mode == "full", "diagnostic modes only for fp8/f16"
        assert (2 * D_DR) % JO_GROUP == 0
    n_dr = 2 * D_DR if hyb else 0  # leading j-tiles handled by e4m3 DoubleRow
    ns_dr = n_dr // JO_GROUP  # stripes that are entirely DoubleRow
    if fp8:
        assert JO_GROUP % 2 == 0, "fp8 layout pairs two j-tiles per dram row"
        # SBUF a-tiles stay uint8 in hyb mode; slices are bitcast per matmul.
        a_dram_dt = mybir.dt.uint8
        a_sb_dt = mybir.dt.uint8 if hyb else mybir.dt.float8e3
    else:
        a_dram_dt = a_sb_dt = mybir.dt.float16

    nc = bass.Bass()
    if fp8:
        # Two j-tiles per 2 KiB dram row: row r = jo*(JO_GROUP/2*128) + tp*128 + p
        # holds tiles (2tp, 2tp+1) for partition p -- same 2 KiB-line /
        # 2 KiB-partition-stride address pattern as the f16 layout (which
        # sustains ~363 GB/s; a 64 KiB power-of-two partition stride does not).
        at = nc.dram_tensor("at", [N // 2, 2 * M_LOC], a_dram_dt, kind="ExternalInput")
    else:
        at = nc.dram_tensor("at", [N, M_LOC], a_dram_dt, kind="ExternalInput")
    b = nc.dram_tensor("b", [P, (KT - n_dr) * H], mybir.dt.float16, kind="ExternalInput")
    if hyb:
        b8 = nc.dram_tensor("b8", [P, D_DR * 256], mybir.dt.uint8, kind="ExternalInput")
    # f16 output staging halves the per-rep writeback traffic; the rounding
    # (~2e-4 relative) is negligible vs the fp8 quantization error.
    out_dt = mybir.dt.float16 if (OUT16 and fp8 and not hyb) else mybir.dt.float32
    ct = nc.dram_tensor("ct", [H, M_LOC], out_dt, kind="ExternalOutput")

    def a_stripe_view(jo):
        if fp8:
            rows = JO_GROUP // 2 * P  # dram rows per stripe
            return (
                at[jo * rows : (jo + 1) * rows, :]
                .rearrange("(tp p) (t2 i) -> p tp t2 i", p=P, i=M_LOC)
                .bitcast(a_sb_dt)
            )
        return (
            at[jo * JO_GROUP * P : (jo + 1) * JO_GROUP * P, :]
            .rearrange("(t p) i -> p t i", p=P)
            .bitcast(a_sb_dt)
        )

    with TileContext(nc) as tc:
        with (
            tc.tile_pool(name="bpool", bufs=1) as bpool,
            tc.tile_pool(name="apool", bufs=A_BUFS) as apool,
            tc.tile_pool(name="opool", bufs=1) as opool,
            tc.tile_pool(name="psum", bufs=1, space="PSUM") as psum_pool,
        ):
            # All of B resident in SBUF, host-prepacked to [128, KT*H] so
            # the load is one fully-contiguous-per-partition transfer.
            b_sb = bpool.tile([P, KT - n_dr, H], mybir.dt.float16)
            nc.sync.dma_start(b_sb[:], b[:, :].rearrange("p (t h) -> p t h", h=H))
            if hyb:
                # DoubleRow stationaries: per pair, [2 k-subtiles, 128] where
                # cols 0-63 = e4m3(B) rows, 64-127 = e4m3(16*(B - hi)).
                b8_sb = bpool.tile([P, D_DR, 2, 128], mybir.dt.float8e4)
                nc.scalar.dma_start(
                    b8_sb[:],
                    b8[:, :]
                    .rearrange("p (pr t2 m) -> p pr t2 m", t2=2, m=128)
                    .bitcast(mybir.dt.float8e4),
                )

            # Output staging tiles, double-buffered by rep parity so the DVE
            # copies of rep r+1 don't wait on rep r's ct writeback DMA.
            out_sbs = [
                opool.tile([H, M_LOC], out_dt, name=f"out{par}")
                for par in range(2)
            ]

            a_shape = [P, JO_GROUP // 2, 2, M_LOC] if fp8 else [P, JO_GROUP, M_LOC]

            def a_slice(a_sb, t, lo, hi):
                if hyb:
                    return a_sb[:, t // 2, t % 2, lo:hi].bitcast(mybir.dt.float8e3)
                if fp8:
                    return a_sb[:, t // 2, t % 2, lo:hi]
                return a_sb[:, t, lo:hi]

            a_res = None
            if mode == "pe":
                a_res = apool.tile(a_shape, a_sb_dt, tag="a_sb", name="a_res")
                nc.sync.dma_start(a_res[:], a_stripe_view(0))

            for rep in range(reps):
                out_sb = out_sbs[rep % 2]
                out_prev = out_sbs[(rep - 1) % 2]
                # Psum tiles double-buffered by rep parity: rep r+1 opens its
                # accumulation group in the other banks while rep r's DVE
                # copies drain.
                psums = (
                    []
                    if mode == "dma"
                    else [
                        psum_pool.tile(
                            [H, F],
                            mybir.dt.float32,
                            # hyb needs the banks for ps_dr: single-buffered
                            tag=f"ps{i}{0 if hyb else rep % 2}",
                            name=f"ps{i}_{rep}",
                        )
                        for i in range(IT)
                    ]
                )
                ps_dr = (
                    [
                        psum_pool.tile(
                            [P, F],
                            mybir.dt.float32,
                            tag=f"psdr{i}",
                            name=f"psdr{i}_{rep}",
                        )
                        for i in range(IT)
                    ]
                    if hyb
                    else []
                )

                # Warm-up matmul: absorbs cross-engine deps (B-load DMA on
                # rep 0; previous rep's DVE copies after) into PE program
                # order, so every real matmul carries at most one sem wait
                # (walrus rejects "too many sync wait commands").
                warm_ps = psum_pool.tile(
                    [H, F], mybir.dt.float32, tag="warm_ps", name=f"warm_ps_{rep}"
                )
                if rep == 0 or mode == "dma":
                    nc.tensor.matmul(
                        warm_ps[:, :H],
                        b_sb[:, 0, :],
                        b_sb[:, 0, :],
                        start=True,
                        stop=True,
                    )
                else:
                    nc.tensor.matmul(
                        warm_ps[:, :H],
                        out_prev[:, :H],
                        out_prev[:, :H],
                        start=True,
                        stop=True,
                    )

                for jo in range(KT // JO_GROUP):
                    if mode == "pe":
                        a_sb = a_res
                    else:
                        a_sb = apool.tile(
                            a_shape,
                            a_sb_dt,
                            tag="a_sb",
                            name=f"a_sb_{rep}",
                        )
                        # Spread loads across issuing paths (SP/ACT HWDGE
                        # rings, optionally SWDGE) so per-transfer completion
                        # gaps overlap across streams.
                        dma_eng = [nc.sync, nc.scalar, nc.gpsimd][jo % N_STREAMS]
                        dma_eng.dma_start(a_sb[:], a_stripe_view(jo))
                    if hyb and jo < ns_dr:
                        # e4m3 DoubleRow: one matmul covers two k-tiles at
                        # 0.5 cycles/row; psum rows 0-63 = B-hi, 64-127 = B-lo.
                        for tp in range(JO_GROUP // 2):
                            pair = jo * (JO_GROUP // 2) + tp
                            for i in range(IT):
                                nc.tensor.matmul(
                                    ps_dr[i],
                                    b8_sb[:, pair, :, :],
                                    a_sb[:, tp, :, i * F : (i + 1) * F].bitcast(
                                        mybir.dt.float8e4
                                    ),
                                    start=(pair == 0),
                                    stop=(pair == D_DR - 1),
                                    perf_mode=mybir.MatmulPerfMode.DoubleRow,
                                )
                        continue
                    for t in range(JO_GROUP):
                        j = jo * JO_GROUP + t
                        if mode == "dma":
                            if t == 0:
                                nc.tensor.matmul(
                                    warm_ps,
                                    b_sb[:, j, :],
                                    a_slice(a_sb, 0, 0, F),
                                    start=True,
                                    stop=True,
                                )
                            continue
                        lhsT = b_sb[:, j - n_dr, :]
                        for i in range(IT):
                            nc.tensor.matmul(
                                psums[i],
                                lhsT,
                                a_slice(a_sb, t, i * F, (i + 1) * F),
                                start=(j == n_dr),
                                stop=(j == KT - 1),
                                perf_mode=(
                                    mybir.MatmulPerfMode.DoublePixel
                                    if DP_MODE and fp8
                                    else None
                                ),
                            )

                if mode == "dma":
                    if rep == reps - 1:
                        for i in range(IT):
                            nc.vector.tensor_copy(
                                out_sb[:, i * F : (i + 1) * F], warm_ps[:]
                            )
                        nc.sync.dma_start(ct[:, :], out_sb[:])
                else:
                    for i in range(IT):
                        half = out_sb[:, i * F : (i + 1) * F]
                        nc.vector.tensor_copy(half, psums[i][:])
                        if hyb:
                            nc.vector.scalar_tensor_tensor(
                                half,
                                ps_dr[i][0:H, :],
                                1.0,
                                half,
                                mybir.AluOpType.mult,
                                mybir.AluOpType.add,
                            )
                            nc.vector.scalar_tensor_tensor(
                                half,
                                ps_dr[i][H : 2 * H, :],
                                1.0 / 16.0,
                                half,
                                mybir.AluOpType.mult,
                                mybir.AluOpType.add,
                            )
                    # Write the result to dram every rep so no rep's compute
                    # is dead in the unrolled timing NEFF (honest slope).
                    nc.sync.dma_start(ct[:, :], out_sb[:])

    _prune_redundant_waits(nc)
    return nc


def _prune_redundant_waits(nc):
    """Transitive reduction of Tile's per-instruction sem waits.

    Walrus rejects instructions with more than one sync-wait command, but
    Tile's sem assignment is not transitively minimal: a slot-recycling DMA
    waits on both {PE >= k} (readers done) and {DMAHW_j >= v} (old write
    done) even though the PE instructions counted by PE>=k themselves waited
    on DMAHW_j >= v.  For a straight-line program, a wait W is implied by a
    co-located wait W0 if some instruction whose completion is counted by W0
    itself waits for W (at >= W's value): drop W then.
    """
    import concourse.mybir as mybir

    insts = []
    for f in nc.m.functions:
        for blk in f.blocks:
            insts.extend(blk.instructions)

    sem_updates = {}  # sem id -> [(cumulative value after this inst, inst)]
    cum = {}
    for inst in insts:
        si = inst.sync_info
        if si is None:
            continue
        for u in si.on_update or []:
            c = cum.get(u.id, 0) + (u.update_value or 1)
            cum[u.id] = c
            sem_updates.setdefault(u.id, []).append((c, inst))

    # eff[inst name] = {sem id: floor} of sem values known to hold once the
    # instruction completes (own waits, closed transitively to fixpoint).
    eff = {}
    own = {}
    for inst in insts:
        si = inst.sync_info
        d = {}
        if si is not None:
            for w in si.on_wait or []:
                d[w.id] = max(d.get(w.id, -1), w.wait_value)
        own[inst.name] = dict(d)
        eff[inst.name] = d

    changed = True
    while changed:
        changed = False
        for inst in insts:
            d = eff[inst.name]
            for sid, v in list(d.items()):
                for c, x in sem_updates.get(sid, []):
                    if c > v:
                        break
                    for s2, v2 in eff[x.name].items():
                        if d.get(s2, -1) < v2:
                            d[s2] = v2
                            changed = True

    n_pruned = 0
    multi_insts = set()
    for inst in insts:
        si = inst.sync_info
        if si is None or not si.on_wait or len(si.on_wait) <= 1:
            continue
        waits = list(si.on_wait)
        keep = []
        for w in waits:
            implied = False
            for w0 in waits:
                if w0 is w or implied:
                    continue
                for c, x in sem_updates.get(w0.id, []):
                    if c > w0.wait_value:
                        break
                    if eff[x.name].get(w.id, -1) >= w.wait_value:
                        implied = True
                        break
            if not implied:
                keep.append(w)
        if len(keep) < len(waits):
            n_pruned += len(waits) - len(keep)
            inst.sync_info = mybir.SyncInfo(
                on_wait=keep, on_update=list(si.on_update or [])
            )
        if len(keep) > 1:
            multi_insts.add(inst.name)

    # Spill fallback: walrus accepts only one sync-wait command per
    # instruction.  For irreducible multi-waits, keep one wait on the
    # instruction and move the rest onto same-engine NOPs inserted just
    # before it (sequencer program order makes them gate the instruction).
    if multi_insts:
        for f in nc.m.functions:
            for blk in f.blocks:
                cur = list(blk.instructions)
                if not any(i.name in multi_insts for i in cur):
                    continue
                new = []
                for inst in cur:
                    if inst.name in multi_insts:
                        waits = list(inst.sync_info.on_wait)
                        for k, w in enumerate(waits[:-1]):
                            new.append(
                                mybir.InstNoOp(
                                    name=f"{inst.name}-wspill{k}",
                                    engine=inst.engine,
                                    bass_nofuse=True,
                                    sync_info=mybir.SyncInfo(
                                        on_wait=[w], on_update=[]
                                    ),
                                )
                            )
                        inst.sync_info = mybir.SyncInfo(
                            on_wait=[waits[-1]],
                            on_update=list(inst.sync_info.on_update or []),
                        )
                    new.append(inst)
                if len(new) != len(cur):
                    blk.instructions = new
    return n_pruned


def get_nc(in_dt=None, reps=1, mode="full"):
    if in_dt is None:
        in_dt = DEFAULT_DT
    key = ("nc", in_dt, reps, mode)
    if key not in _CACHE:
        _CACHE[key] = _build_nc(in_dt, reps, mode)
    return _CACHE[key]


def make_in_maps(location_data, motion_data, in_dt=None):
    if in_dt is None:
        in_dt = DEFAULT_DT
    import ml_dtypes

    A = np.asarray(location_data, dtype=np.float32)
    B = np.asarray(motion_data, dtype=np.float32)
    assert A.shape == (N, N) and B.shape == (N, H)
    hyb = in_dt == "hyb"
    fp8 = in_dt == "fp8" or hyb
    n_dr = 2 * D_DR if hyb else 0
    kcut = n_dr * P  # j below this handled by e4m3 DoubleRow
    b_scale = (1.0 / A_SCALE) if fp8 else 1.0
    # Pack B so row j = t*128 + p lands at b_packed[p, t*H:(t+1)*H]:
    # the device-side load becomes contiguous per partition.
    b_packed = np.ascontiguousarray(
        (B[kcut:] * b_scale)
        .reshape(KT - n_dr, P, H)
        .transpose(1, 0, 2)
        .reshape(P, (KT - n_dr) * H),
        dtype=np.float16,
    )
    b8_packed = None
    if hyb:
        E4 = ml_dtypes.float8_e4m3
        Bhi = np.asarray(B[:kcut], dtype=E4)
        Blo = np.asarray((B[:kcut] - Bhi.astype(np.float32)) * 16.0, dtype=E4)
        St = np.zeros((kcut, 128), dtype=E4)  # hi | lo stationary cols
        St[:, :H] = Bhi
        St[:, H:2 * H] = Blo
        b8_packed = np.ascontiguousarray(
            St.view(np.uint8).reshape(D_DR, 2, P, 128).transpose(2, 0, 1, 3)
        ).reshape(P, D_DR * 256)
    in_maps = []
    for c in range(NCORES):
        As = A[c * M_LOC : (c + 1) * M_LOC, :]  # [1024 i, 8192 j]
        if fp8:
            q = np.asarray(As * A_SCALE, dtype=ml_dtypes.float8_e3m4).view(np.uint8)
            if hyb:
                q4 = np.asarray(As[:, :kcut], dtype=ml_dtypes.float8_e4m3).view(
                    np.uint8
                )
                q = np.concatenate([q4, q[:, kcut:]], axis=1)
            # dram row tpg*128 + p, col t2*1024 + i  <-  q[i, (2*tpg+t2)*128 + p]
            at_c = np.ascontiguousarray(
                q.reshape(M_LOC, KT // 2, 2, P).transpose(1, 3, 2, 0)
            ).reshape(N // 2, 2 * M_LOC)
            m = {"at": at_c, "b": b_packed}
            if hyb:
                m["b8"] = b8_packed
            in_maps.append(m)
        else:
            at_c = np.ascontiguousarray(As.T, dtype=np.float16)
            in_maps.append({"at": at_c, "b": b_packed})
    return in_maps


def assemble_output(results):
    return np.concatenate([np.asarray(r["ct"]).T for r in results], axis=0)


def kernel(location_data, motion_data):
    from concourse.bass_utils import run_bass_kernel_spmd

    nc = get_nc(in_dt=DEFAULT_DT)
    in_maps = make_in_maps(location_data, motion_data, in_dt=DEFAULT_DT)
    res = run_bass_kernel_spmd(nc, in_maps, core_ids=list(range(NCORES)))
    return assemble_output(res.results).astype(np.float32)
